# revision 4
# baseline (speedup 1.0000x reference)
"""Bass/Trainium2 kernel for nn_BilinearInteraction.

Computes out[b, p, :] = (x[b, i_p, :] @ W[p].T + bias[p]) * x[b, j_p, :]
for the 325 upper-triangular field pairs (i_p < j_p), batch B=4096,
F=26 fields, D=32 embed dim.

Strategy (data parallel over 8 NeuronCores, 512 batch rows each):
  - batch rows live on SBUF partitions (4 tiles of 128 rows per core).
  - pairs sharing the same i-field are contiguous in p, so for each field
    f the output columns [pstart(f)*32, (pstart(f)+25-f)*32) are produced
    by ONE stationary operand: xT_f = transpose(x[:, f, :]) augmented
    with a row of ones (bias trick -> K=33 contraction).
  - fields are stacked 3-at-a-time into block-diagonal packed weights
    (K=99) so one PE transpose + one psum->sbuf copy serves 3 fields.
  - weights are packed on the host into wbs[99, 10400]:
      wbs[33*(f%3)+d, (pstart(f)+(j-f-1))*32+e] = W[p(f,j), e, d]
      wbs[33*(f%3)+32, ...] = bias
  - matmuls run as float32r (full-rate fp32 path on the PE array).
  - VectorE multiplies psum by v_j (a natural column slice of x) into a
    [128, 10400] staging tile; one 5.3 MB DMA per batch tile writes HBM.
"""

import os
import sys

import numpy as np

for _p in (
    "/root/.axon_site",
    "/root/.axon_site/_ro/trn_rl_repo",
    "/root/.axon_site/_ro/pypackages",
    "/opt/trn_rl_repo",
):
    if os.path.isdir(_p) and _p not in sys.path:
        sys.path.append(_p)

import concourse.bass as bass
import concourse.bacc as bacc
import concourse.tile as tile
from concourse import mybir
from concourse.bass_utils import run_bass_kernel_spmd
from concourse.masks import make_identity

N_CORES = 8
B, F, D = 4096, 26, 32
NPAIR = 325  # F*(F-1)/2
BLOC = B // N_CORES  # 512 batch rows per core
PB = 128  # batch rows per tile (partition dim)
NT = BLOC // PB  # 4 tiles per core
OUTW = NPAIR * D  # 10400 output columns
DA = D + 1  # field block width in augmented x (32 data + 1 one)

FP32 = mybir.dt.float32
FP32R = mybir.dt.float32r

# matmul input dtype: "f32r" (full-rate fp32 PE path) or "f32" (exact, 4x slower)
MM_MODE = os.environ.get("BILIN_MM_MODE", "f32r")

# fields grouped 3 at a time for block-diagonal matmuls (f=24 alone)
GROUPS = [tuple(range(g, min(g + 3, 25))) for g in range(0, 25, 3)]


def _pstart(f: int) -> int:
    # first pair index whose i == f (pairs sorted by (i, j))
    return 25 * f - f * (f - 1) // 2


def _nf(f: int) -> int:
    return (25 - f) * D  # output columns owned by field f


def _group_width(fs) -> int:
    return sum(_nf(f) for f in fs)


def _chunks(width: int):
    """Split width into psum chunks, each <=512, >=256 where possible, %32==0."""
    out = []
    rem = width
    while rem > 0:
        if rem <= 512:
            c = rem
        elif rem >= 768:
            c = 512
        else:
            c = rem - 256
        out.append(c)
        rem -= c
    starts = []
    s = 0
    for c in out:
        starts.append((s, s + c))
        s += c
    return starts


def pack_weights(W: np.ndarray, b: np.ndarray) -> np.ndarray:
    """Build wbs[99, OUTW] with 3-phase block-diagonal layout + bias rows."""
    W = np.ascontiguousarray(W, dtype=np.float32)
    b = np.ascontiguousarray(b, dtype=np.float32)
    wbs = np.zeros((99, OUTW), dtype=np.float32)
    for f in range(25):
        ph = f % 3
        p0 = _pstart(f)
        npair = 25 - f
        base = p0 * D
        blk = W[p0 : p0 + npair].transpose(2, 0, 1).reshape(D, npair * D)
        wbs[33 * ph : 33 * ph + D, base : base + npair * D] = blk
        wbs[33 * ph + D, base : base + npair * D] = b[p0 : p0 + npair].reshape(-1)
    return wbs


def _emit(tc: tile.TileContext, out_ap, xs_ap, wbs_ap):
    from contextlib import ExitStack

    nc = tc.nc
    mm_dt = FP32R if MM_MODE == "f32r" else FP32

    with ExitStack() as ctx:
        const = ctx.enter_context(tc.tile_pool(name="const", bufs=1))
        xp = ctx.enter_context(tc.tile_pool(name="xp", bufs=2))
        stg = ctx.enter_context(tc.tile_pool(name="stg", bufs=2))
        op = ctx.enter_context(tc.tile_pool(name="op", bufs=2))
        tps = ctx.enter_context(tc.tile_pool(name="tps", bufs=2, space="PSUM"))
        mps = ctx.enter_context(tc.tile_pool(name="mps", bufs=6, space="PSUM"))

        ident = const.tile([128, 128], FP32, tag="ident", name="ident")
        make_identity(nc, ident)
        wbs_sb = const.tile([99, OUTW], mm_dt, tag="wbs", name="wbs_sb")
        nc.sync.dma_start(out=wbs_sb, in_=wbs_ap)

        for bt in range(NT):
            b0 = bt * PB
            # x tile with a column of ones appended to each field block
            x_aug = xp.tile([PB, F, DA], FP32, tag="xaug", name=f"xaug{bt}")
            nc.sync.dma_start(out=x_aug[:, :, 0:D], in_=xs_ap[b0 : b0 + PB])
            nc.gpsimd.memset(x_aug[:, :, D : D + 1], 1.0)

            # transpose each field group -> [33*len(fs), 128] staging (PE + ACT)
            stgs = []
            for gi, fs in enumerate(GROUPS):
                kg = 33 * len(fs)
                tin = x_aug[:, fs[0] : fs[0] + len(fs), :].rearrange(
                    "p a b -> p (a b)"
                )
                ps_t = tps.tile([kg, PB], FP32, tag="tp", name=f"pst{bt}_{gi}")
                nc.tensor.transpose(ps_t, tin, ident)
                stg_g = stg.tile([kg, PB], mm_dt, tag=f"sg{gi}", name=f"stg{bt}_{gi}")
                nc.scalar.copy(stg_g, ps_t)
                stgs.append(stg_g)

            out_sb = op.tile([PB, OUTW], FP32, tag="osb", name=f"osb{bt}")

            for gi, fs in enumerate(GROUPS):
                kg = 33 * len(fs)
                gbase = _pstart(fs[0]) * D  # group's first output column
                width = _group_width(fs)
                # field offsets within the group's column span
                offs = []
                o = 0
                for f in fs:
                    offs.append(o)
                    o += _nf(f)
                for c0, c1 in _chunks(width):
                    ps_m = mps.tile(
                        [PB, c1 - c0], FP32, tag="mp", name=f"psm{bt}_{gi}_{c0}"
                    )
                    nc.tensor.matmul(
                        ps_m,
                        stgs[gi],
                        wbs_sb[:kg, gbase + c0 : gbase + c1],
                        start=True,
                        stop=True,
                    )
                    # multiply by v_j per field span inside this chunk
                    for f, off in zip(fs, offs):
                        s0 = max(c0, off)
                        s1 = min(c1, off + _nf(f))
                        if s0 >= s1:
                            continue
                        j0 = f + 1 + (s0 - off) // D
                        nj = (s1 - s0) // D
                        nc.vector.tensor_mul(
                            out_sb[:, gbase + s0 : gbase + s1],
                            ps_m[:, s0 - c0 : s1 - c0],
                            x_aug[:, j0 : j0 + nj, 0:D],
                        )

            nc.sync.dma_start(out=out_ap[b0 : b0 + PB, :], in_=out_sb)


_CACHE = {}


def _build():
    if "nc" in _CACHE:
        return _CACHE["nc"]
    nc = bacc.Bacc("TRN2", target_bir_lowering=False, debug=False)
    xs = nc.dram_tensor("xs", [BLOC, F, D], FP32, kind="ExternalInput").ap()
    mm_dt = FP32R if MM_MODE == "f32r" else FP32
    wbs = nc.dram_tensor("wbs", [99, OUTW], mm_dt, kind="ExternalInput").ap()
    out = nc.dram_tensor("out", [BLOC, OUTW], FP32, kind="ExternalOutput").ap()
    with tile.TileContext(nc) as tc:
        _emit(tc, out, xs, wbs)
    nc.compile()
    _CACHE["nc"] = nc
    return nc


def run(
    x: np.ndarray,
    W: np.ndarray,
    b: np.ndarray,
    trace: bool = False,
    tmpdir: str | None = None,
):
    """Shard, execute on 8 cores, gather. Returns (out, results_obj)."""
    x = np.ascontiguousarray(x, dtype=np.float32)
    wbs = pack_weights(W, b)
    nc = _build()
    in_maps = [
        {"xs": x[c * BLOC : (c + 1) * BLOC], "wbs": wbs} for c in range(N_CORES)
    ]
    res = run_bass_kernel_spmd(
        nc, in_maps, core_ids=list(range(N_CORES)), trace=trace, tmpdir=tmpdir
    )
    parts = [res.results[c]["out"].reshape(BLOC, NPAIR, D) for c in range(N_CORES)]
    out = np.concatenate(parts, axis=0).astype(np.float32, copy=False)
    return out, res


def kernel(x: np.ndarray, W: np.ndarray, b: np.ndarray) -> np.ndarray:
    out, _ = run(x, W, b, trace=False)
    return out


if __name__ == "__main__":
    rng = np.random.default_rng(0)
    x = rng.standard_normal((B, F, D), dtype=np.float32)
    W = rng.standard_normal((NPAIR, D, D), dtype=np.float32) / np.sqrt(D)
    b = rng.standard_normal((NPAIR, D), dtype=np.float32) * 0.01
    out = kernel(x, W, b)
    print("out", out.shape, out.dtype)
